# revision 2
# baseline (speedup 1.0000x reference)
"""Trainium2 Bass kernel for nn_AInnoFaceLoss (IoU -> argmax -> gather).

reference semantics:
    a = xyxy(anchors); g = xyxy(ground_truth)
    iou[b, i, j] over (A=100000 anchors) x (G=64 gt) per batch (B=4)
    target_score = iou.max(-1); target_bbox = g[b, argmax(-1)]
(fs_proposal / ss_proposal are unused by the reference.)

Strategy (8 NeuronCores, data-parallel over anchors):
  - Host sorts anchors by x into 8 strips (one per core); within a strip,
    anchors are sorted by y into NB blocks. Each (core, block, batch) only
    ever overlaps a small subset of the 64 gt boxes (computed exactly on the
    host; global gt 0 is always included so all-zero-IoU rows reproduce the
    reference argmax tie-break). Subsets are padded to G_PAD with far-away
    dummy boxes.
  - Per anchor-tile (128 anchors) one bf16 matmul with hi/mid/lo split
    operands (exact to fp32) emits [Ux|Vx|Uy|Vy|AREA] where
    U = ax2-gx1, V = gx2-ax1 (per axis) and AREA = area_a + area_g.
  - DVE computes inter_w = relu(min(U, wa, V, wg)) per axis, inter, and
    r = inter/(area_a+area_g)  (iou = r/(1-r), monotone), then a segmented
    max and a first-index argmax via max(mask * (G-j)).
  - The per-(tile,batch) argmax is PE-transposed, broadcast to (b, j)
    partitions with selector matmuls, turned into a transposed one-hot, and
    two small accumulating matmuls against hi/mid coordinate tables gather
    the best gt box per anchor. score = rmax/(1-rmax).

The full inputs arrive here; sharding / gt-subset tables are prepared on the
host with numpy, the compiled NEFF runs SPMD on cores 0-7 via
run_bass_kernel_spmd, and outputs are un-permuted and assembled on the host.
"""

import sys
import numpy as np
from contextlib import ExitStack

for _p in ("/opt/trn_rl_repo", "/root/.axon_site/_ro/trn_rl_repo"):
    if _p not in sys.path:
        sys.path.append(_p)

from concourse import bacc, mybir, tile  # noqa: E402
from concourse.bass_utils import run_bass_kernel_spmd  # noqa: E402

F32 = mybir.dt.float32
BF16 = mybir.dt.bfloat16
AL = mybir.AluOpType
AF = mybir.ActivationFunctionType

N_CORES = 8
B = 4
NT = 104          # anchor tiles per core (A_loc = 13312 >= 12500)
NB = 13           # y-blocks per core
CH = 4            # tiles per DVE chunk
A_FULL = 100000
G_FULL = 64


def _bf16_round(x):
    """Round float32 -> bfloat16 (round-to-nearest-even), keep float32 type."""
    u = np.asarray(x, np.float32).view(np.uint32)
    rounded = (u + 0x7FFF + ((u >> 16) & 1)).astype(np.uint32) & 0xFFFF0000
    return rounded.view(np.float32)


def _to_bf16(x):
    """float32 array -> bfloat16 array (ml_dtypes if present, else uint16 bits)."""
    try:
        import ml_dtypes
        return np.asarray(x, np.float32).astype(ml_dtypes.bfloat16)
    except ImportError:
        u = np.asarray(x, np.float32).view(np.uint32)
        rounded = (u + 0x7FFF + ((u >> 16) & 1)).astype(np.uint32) & 0xFFFF0000
        return (rounded >> 16).astype(np.uint16)


def split_hml(x):
    x = np.asarray(x, np.float32)
    h = _bf16_round(x)
    m = _bf16_round(x - h)
    l = _bf16_round(x - h - m)
    return h, m, l


def build_graph(NT, B, G, CH, NB, n_cores=8):
    FB = B * G
    W = 5 * FB
    assert W <= 512
    A_loc = NT * 128
    TPB = NT // NB
    assert NT % NB == 0 and TPB % CH == 0
    NCH = NT // CH
    GP = B * G
    assert GP <= 128
    M = B * 4
    NTB = ((NT * 32 + 127) // 128) * 128 // 32
    NBLK = NTB * 32 // 128

    nc = bacc.Bacc("TRN2", target_bir_lowering=False, debug=False,
                   num_devices=n_cores)

    def din(name, shape, dt=F32):
        return nc.dram_tensor(name, list(shape), dt, kind="ExternalInput")

    lhs_d = din("lhs", [18, NT * 128], BF16)
    rhs_d = din("rhs", [18, NB * W], BF16)
    wab_d = din("wab", [128, NT])
    hab_d = din("hab", [128, NT])
    wgx_d = din("wgx", [128, NB * FB])
    wgy_d = din("wgy", [128, NB * FB])
    iot_d = din("iotarev", [128, FB], BF16)
    iotp_d = din("iotarev_part", [128, 1])
    esel_d = din("esel", [128, 4 * 128], BF16)
    ident_d = din("ident", [128, 128], BF16)
    gtbl_d = din("gtbl", [128, NB * 2 * M], BF16)

    score_out = nc.dram_tensor("score", [B, A_loc], F32, kind="ExternalOutput")
    bbox_out = nc.dram_tensor("bbox", [M, A_loc], F32, kind="ExternalOutput")

    with tile.TileContext(nc) as tc, ExitStack() as ctx:
        sb = ctx.enter_context(tc.tile_pool(name="sb", bufs=1))

        rhs = sb.tile([18, NB * W], BF16)
        wab = sb.tile([128, NT], F32)
        hab = sb.tile([128, NT], F32)
        wgx = sb.tile([128, NB * FB], F32)
        wgy = sb.tile([128, NB * FB], F32)
        iot = sb.tile([128, FB], BF16)
        iotp = sb.tile([128, 1], F32)
        esel = sb.tile([128, 4 * 128], BF16)
        ident = sb.tile([128, 128], BF16)
        gtbl = sb.tile([128, NB * 2 * M], BF16)
        for dst, src in [(rhs, rhs_d), (wab, wab_d), (hab, hab_d),
                         (wgx, wgx_d), (wgy, wgy_d), (iot, iot_d),
                         (iotp, iotp_d), (esel, esel_d), (ident, ident_d),
                         (gtbl, gtbl_d)]:
            nc.sync.dma_start(dst[:], src[:])

        rmax_sb = sb.tile([128, NT * B], F32)
        jrev32 = sb.tile([128, NTB * 32], BF16)
        score_sb = sb.tile([128, NT * B], F32)
        jrevT = sb.tile([128, NBLK * 128], BF16)
        oht = sb.tile([128, A_loc], BF16)
        bbs = sb.tile([M, A_loc], F32)
        nc.vector.memset(jrev32[:], 0.0)

        # ---------------- phase 1: IoU ratio + segmented argmax ----------
        with tc.tile_pool(name="ps", bufs=2, space="PSUM") as ps, \
             tc.tile_pool(name="lp", bufs=3) as lp, \
             tc.tile_pool(name="wk", bufs=2) as wk:
            for c in range(NCH):
                t0 = c * CH
                blk = t0 // TPB
                lhs = lp.tile([18, CH * 128], BF16, tag="lhs")
                nc.sync.dma_start(lhs[:], lhs_d[:, t0 * 128:(t0 + CH) * 128])
                uv = ps.tile([128, CH, 512], F32, tag="uv")
                for k in range(CH):
                    nc.tensor.matmul(uv[:, k, 0:W], lhs[:, k * 128:(k + 1) * 128],
                                     rhs[:, blk * W:(blk + 1) * W],
                                     start=True, stop=True)

                t1x = wk.tile([128, CH, FB], F32, tag="t1x")
                t2x = wk.tile([128, CH, FB], F32, tag="t2x")
                t1y = wk.tile([128, CH, FB], F32, tag="t1y")
                t2y = wk.tile([128, CH, FB], F32, tag="t2y")
                wa_v = wab[:, t0:t0 + CH, None].to_broadcast((128, CH, FB))
                ha_v = hab[:, t0:t0 + CH, None].to_broadcast((128, CH, FB))
                wgx_v = wgx[:, None, blk * FB:(blk + 1) * FB].to_broadcast(
                    (128, CH, FB))
                wgy_v = wgy[:, None, blk * FB:(blk + 1) * FB].to_broadcast(
                    (128, CH, FB))
                nc.vector.tensor_tensor(t1x[:], uv[:, :, 0:FB], wa_v, AL.min)
                nc.vector.tensor_tensor(t2x[:], uv[:, :, FB:2 * FB], wgx_v, AL.min)
                nc.vector.tensor_tensor(t1y[:], uv[:, :, 2 * FB:3 * FB], ha_v, AL.min)
                nc.vector.tensor_tensor(t2y[:], uv[:, :, 3 * FB:4 * FB], wgy_v, AL.min)
                rc = wk.tile([128, CH, FB], F32, tag="rc")
                nc.vector.reciprocal_approx_fast(rc[:], uv[:, :, 4 * FB:5 * FB])

                mx = wk.tile([128, CH, FB], F32, tag="mx")
                my = wk.tile([128, CH, FB], F32, tag="my")
                nc.vector.tensor_tensor(mx[:], t1x[:], t2x[:], AL.min)
                nc.vector.tensor_tensor(my[:], t1y[:], t2y[:], AL.min)
                relu_my = wk.tile([128, CH, FB], F32, tag="relu_my")
                nc.scalar.activation(relu_my[:], my[:], AF.Relu)
                inter = wk.tile([128, CH, FB], F32, tag="inter")
                nc.vector.scalar_tensor_tensor(inter[:], mx[:], 0.0, relu_my[:],
                                               AL.max, AL.mult)
                r = wk.tile([128, CH, FB], F32, tag="r")
                nc.vector.tensor_tensor(r[:], inter[:], rc[:], AL.mult)

                r4 = r[:].rearrange("p c (b j) -> p c b j", j=G)
                rm = rmax_sb[:, t0 * B:(t0 + CH) * B].rearrange(
                    "p (c b) -> p c b", b=B)
                nc.vector.tensor_reduce(rm, r4, mybir.AxisListType.X, AL.max)
                mask = wk.tile([128, CH, B, G], BF16, tag="mask")
                rm_v = rm[:, :, :, None].to_broadcast((128, CH, B, G))
                nc.vector.tensor_tensor(mask[:], r4, rm_v, AL.is_ge)
                mi = wk.tile([128, CH, B, G], BF16, tag="mi")
                iot_v = iot[:].rearrange("p (b j) -> p b j", j=G)[
                    :, None, :, :].to_broadcast((128, CH, B, G))
                nc.vector.tensor_tensor(mi[:], mask[:], iot_v, AL.mult)
                jr = jrev32[:, t0 * 32:(t0 + CH) * 32].rearrange(
                    "p (c k) -> p c k", k=32)[:, :, 0:B]
                nc.vector.tensor_reduce(jr, mi[:], mybir.AxisListType.X, AL.max)

        # ---------------- phase 2: score ----------------
        one_minus = sb.tile([128, NT * B], F32)
        nc.vector.tensor_scalar(one_minus[:], rmax_sb[:], -1.0, 1.0,
                                AL.mult, AL.add)
        recip = sb.tile([128, NT * B], F32)
        nc.vector.reciprocal(recip[:], one_minus[:])
        nc.vector.tensor_tensor(score_sb[:], rmax_sb[:], recip[:], AL.mult)
        for b in range(B):
            nc.sync.dma_start(
                score_out[b, :].rearrange("(t i) -> i t", i=128),
                score_sb[:].rearrange("p (t b) -> p t b", b=B)[:, :, b])

        # ---------------- phase 3: transpose jrev ----------------
        with tc.tile_pool(name="pst", bufs=2, space="PSUM") as pst:
            for blk in range(NBLK):
                tp = pst.tile([128, 128], BF16, tag="tp")
                nc.tensor.transpose(tp[:], jrev32[:, blk * 128:(blk + 1) * 128],
                                    ident[:])
                nc.scalar.copy(jrevT[:, blk * 128:(blk + 1) * 128], tp[:])

        # ---------------- phase 4: one-hot + gather ----------------
        with tc.tile_pool(name="psg", bufs=2, space="PSUM") as psg, \
             tc.tile_pool(name="psb", bufs=2, space="PSUM") as psb:
            for g4 in range(0, NT, 4):
                gw = min(4, NT - g4)
                jb = psg.tile([128, 4, 128], F32, tag="jb")
                for k in range(gw):
                    t = g4 + k
                    tm4 = t % 4
                    nc.tensor.matmul(jb[:, k, :], esel[:, tm4 * 128:(tm4 + 1) * 128],
                                     jrevT[:, (t // 4) * 128:(t // 4 + 1) * 128],
                                     start=True, stop=True)
                nc.vector.tensor_scalar(
                    oht[0:GP, g4 * 128:(g4 + gw) * 128],
                    jb[0:GP, 0:gw, :].rearrange("p a b -> p (a b)"),
                    iotp[0:GP, 0:1], None, AL.is_equal)
            AB = TPB * 128
            CW = 512 if AB % 512 == 0 else AB // (AB // 512 + 1)
            for blk in range(NB):
                for q in range(blk * AB, (blk + 1) * AB, CW):
                    bb = psb.tile([M, CW], F32, tag="bb")
                    nc.tensor.matmul(bb[:], gtbl[0:GP, blk * 2 * M:blk * 2 * M + M],
                                     oht[0:GP, q:q + CW], start=True, stop=False)
                    nc.tensor.matmul(
                        bb[:], gtbl[0:GP, blk * 2 * M + M:(blk + 1) * 2 * M],
                        oht[0:GP, q:q + CW], start=False, stop=True)
                    nc.scalar.copy(bbs[:, q:q + CW], bb[:])
        nc.sync.dma_start(bbox_out[:], bbs[:])

    nc.compile()
    return nc


def host_prep(anchors_pad, gt_sub, NT, B, G, NB):
    """anchors_pad: [NT*128, 4] xywh. gt_sub: [NB, B, G, 4] xywh subset tables.
    Returns the per-core input dict (bf16 tensors as uint16 bit patterns)."""
    FB = B * G
    W = 5 * FB
    GP = B * G
    M = B * 4
    A_loc = NT * 128

    ax1 = anchors_pad[:, 0].astype(np.float32)
    ay1 = anchors_pad[:, 1].astype(np.float32)
    wa = anchors_pad[:, 2].astype(np.float32)
    ha = anchors_pad[:, 3].astype(np.float32)
    ax2 = ax1 + wa
    ay2 = ay1 + ha
    aa = wa * ha

    lhs = np.empty((18, A_loc), np.float32)
    lhs[0:3] = np.stack(split_hml(ax2))
    lhs[3:6] = np.stack(split_hml(ax1))
    lhs[6:9] = np.stack(split_hml(ay2))
    lhs[9:12] = np.stack(split_hml(ay1))
    lhs[12:15] = np.stack(split_hml(aa))
    lhs[15:18] = 1.0

    gx1 = gt_sub[..., 0].astype(np.float32)
    gy1 = gt_sub[..., 1].astype(np.float32)
    wg = gt_sub[..., 2].astype(np.float32)
    hg = gt_sub[..., 3].astype(np.float32)
    gx2 = gx1 + wg
    gy2 = gy1 + hg
    ag = wg * hg

    rhs = np.zeros((18, NB, 5, FB), np.float32)
    ones_rows = {0: slice(0, 3), 1: slice(3, 6), 2: slice(6, 9),
                 3: slice(9, 12), 4: slice(12, 15)}
    signs = {0: 1.0, 1: -1.0, 2: 1.0, 3: -1.0, 4: 1.0}
    gvals = {0: -gx1, 1: gx2, 2: -gy1, 3: gy2, 4: ag}
    for part in range(5):
        rhs[ones_rows[part], :, part, :] = signs[part]
        h, m, l = split_hml(gvals[part].reshape(NB, FB))
        rhs[15, :, part, :] = h
        rhs[16, :, part, :] = m
        rhs[17, :, part, :] = l
    rhs = rhs.reshape(18, NB * W)

    wab = wa.reshape(NT, 128).T.copy().astype(np.float32)
    hab = ha.reshape(NT, 128).T.copy().astype(np.float32)
    wgx = np.broadcast_to(wg.reshape(NB, FB)[None], (128, NB, FB)).reshape(
        128, NB * FB).copy()
    wgy = np.broadcast_to(hg.reshape(NB, FB)[None], (128, NB, FB)).reshape(
        128, NB * FB).copy()
    iotarev = np.broadcast_to((G - np.arange(G)).astype(np.float32)[None, :],
                              (B, G)).reshape(FB)
    iotarev = np.broadcast_to(iotarev, (128, FB))
    iotp = np.zeros((128, 1), np.float32)
    p = np.arange(GP)
    iotp[:GP, 0] = G - (p % G)
    esel = np.zeros((128, 4 * 128), np.float32)
    for tm4 in range(4):
        esel[tm4 * 32 + p // G, tm4 * 128 + p] = 1.0
    ident = np.eye(128, dtype=np.float32)

    gxyxy = np.stack([gx1, gy1, gx2, gy2], axis=-1)  # [NB, B, G, 4]
    gh = _bf16_round(gxyxy)
    gm = _bf16_round(gxyxy - gh)
    gtbl = np.zeros((128, NB, 2, M), np.float32)
    bidx = p // G
    for b in range(B):
        sel = p[bidx == b]
        j = sel % G
        for cc in range(4):
            gtbl[sel, :, 0, b * 4 + cc] = gh[:, b, j, cc].T
            gtbl[sel, :, 1, b * 4 + cc] = gm[:, b, j, cc].T
    gtbl = gtbl.reshape(128, NB * 2 * M)

    return {"lhs": _to_bf16(lhs), "rhs": _to_bf16(rhs),
            "wab": wab, "hab": hab, "wgx": wgx, "wgy": wgy,
            "iotarev": _to_bf16(iotarev), "iotarev_part": iotp,
            "esel": _to_bf16(esel), "ident": _to_bf16(ident),
            "gtbl": _to_bf16(gtbl)}


def shard_inputs(anchors, gt, G):
    A = len(anchors)
    A_core = A // N_CORES
    A_loc = NT * 128
    TPB = NT // NB
    ax1 = anchors[:, 0]
    ay1 = anchors[:, 1]
    ax2 = ax1 + anchors[:, 2]
    ay2 = ay1 + anchors[:, 3]
    gx1 = gt[:, :, 0]
    gy1 = gt[:, :, 1]
    gx2 = gx1 + gt[:, :, 2]
    gy2 = gy1 + gt[:, :, 3]

    order = np.argsort(ax1, kind="stable")
    in_maps = []
    perms = []
    dummy = np.array([1e6, 1e6, 10.0, 10.0], np.float32)
    for c in range(N_CORES):
        idx = order[c * A_core:(c + 1) * A_core]
        idx = idx[np.argsort(ay1[idx], kind="stable")]
        perms.append(idx)
        pad = A_loc - len(idx)
        anchors_pad = np.concatenate(
            [anchors[idx], np.tile(dummy[None], (pad, 1))], axis=0)
        gt_sub = np.zeros((NB, B, G, 4), np.float32)
        gt_sub[..., 0] = 1e6
        gt_sub[..., 1] = 1e6
        gt_sub[..., 2] = 10.0
        gt_sub[..., 3] = 10.0
        for nb in range(NB):
            lo, hi = nb * TPB * 128, min((nb + 1) * TPB * 128, len(idx))
            if lo >= len(idx):
                continue
            bidx = idx[lo:hi]
            xlo, xhi = ax1[bidx].min(), ax2[bidx].max()
            ylo, yhi = ay1[bidx].min(), ay2[bidx].max()
            for b in range(B):
                ssel = np.flatnonzero((gx2[b] > xlo) & (gx1[b] < xhi) &
                                      (gy2[b] > ylo) & (gy1[b] < yhi))
                if len(ssel) == 0 or ssel[0] != 0:
                    ssel = np.concatenate([[0], ssel])
                assert len(ssel) <= G, f"gt subset {len(ssel)} exceeds G={G}"
                gt_sub[nb, b, :len(ssel)] = gt[b, ssel]
        in_maps.append(host_prep(anchors_pad, gt_sub, NT, B, G, NB))
    return in_maps, perms


def max_subset_size(anchors, gt):
    """Exact max gt-subset size over (core, block, batch) for this data."""
    A = len(anchors)
    A_core = A // N_CORES
    TPB = NT // NB
    ax1 = anchors[:, 0]
    ay1 = anchors[:, 1]
    ax2 = ax1 + anchors[:, 2]
    ay2 = ay1 + anchors[:, 3]
    gx1 = gt[:, :, 0]
    gy1 = gt[:, :, 1]
    gx2 = gx1 + gt[:, :, 2]
    gy2 = gy1 + gt[:, :, 3]
    order = np.argsort(ax1, kind="stable")
    mx = 1
    for c in range(N_CORES):
        idx = order[c * A_core:(c + 1) * A_core]
        idx = idx[np.argsort(ay1[idx], kind="stable")]
        for nb in range(NB):
            lo, hi = nb * TPB * 128, min((nb + 1) * TPB * 128, len(idx))
            if lo >= len(idx):
                continue
            bidx = idx[lo:hi]
            xlo, xhi = ax1[bidx].min(), ax2[bidx].max()
            ylo, yhi = ay1[bidx].min(), ay2[bidx].max()
            for b in range(B):
                ssel = (gx2[b] > xlo) & (gx1[b] < xhi) & \
                       (gy2[b] > ylo) & (gy1[b] < yhi)
                n = int(ssel.sum()) + (0 if ssel[0] else 1)
                mx = max(mx, n)
    return mx


_GRAPH_CACHE = {}


def _get_graph(G):
    if G not in _GRAPH_CACHE:
        _GRAPH_CACHE[G] = build_graph(NT, B, G, CH, NB, n_cores=N_CORES)
    return _GRAPH_CACHE[G]


def kernel(fs_proposal=None, ss_proposal=None, anchors=None, ground_truth=None,
           **_unused):
    anchors = np.ascontiguousarray(np.asarray(anchors, np.float32))
    gt = np.ascontiguousarray(np.asarray(ground_truth, np.float32))
    assert anchors.shape == (A_FULL, 4) and gt.shape == (B, G_FULL, 4)

    G = max(8, min(25, max_subset_size(anchors, gt)))
    nc = _get_graph(G)
    in_maps, perms = shard_inputs(anchors, gt, G)
    res = run_bass_kernel_spmd(nc, in_maps, core_ids=list(range(N_CORES)))

    A_loc = NT * 128
    score = np.empty((B, A_FULL), np.float32)
    bbox = np.empty((B, A_FULL, 4), np.float32)
    for c in range(N_CORES):
        idx = perms[c]
        n = len(idx)
        score[:, idx] = res.results[c]["score"][:, :n]
        bb = res.results[c]["bbox"].reshape(B, 4, A_loc)
        bbox[:, idx, :] = bb[:, :, :n].transpose(0, 2, 1)
    return score, bbox


# revision 21
# speedup vs baseline: 1.4991x; 1.4991x over previous
"""Trainium2 Bass kernel for nn_AInnoFaceLoss (IoU -> argmax -> gather).

reference semantics:
    a = xyxy(anchors); g = xyxy(ground_truth)
    iou[b, i, j] over (A=100000 anchors) x (G=64 gt) per batch (B=4)
    target_score = iou.max(-1); target_bbox = g[b, argmax(-1)]
(fs_proposal / ss_proposal are unused by the reference.)

Strategy (8 NeuronCores, data-parallel over anchors):
  - Host sorts anchors by x into 8 strips (one per core); within a strip,
    anchors are sorted by y into NB blocks. Each (core, block, batch) only
    ever overlaps a small subset of the 64 gt boxes (computed exactly on the
    host; global gt 0 is always included so all-zero-IoU rows reproduce the
    reference argmax tie-break). Subsets are padded to G_PAD with far-away
    dummy boxes.
  - Per anchor-tile (128 anchors) one bf16 matmul with hi/mid/lo split
    operands (exact to fp32) emits [Ux|Vx|Uy|Vy|AREA] where
    U = ax2-gx1, V = gx2-ax1 (per axis) and AREA = area_a + area_g.
  - DVE computes inter_w = relu(min(U, wa, V, wg)) per axis, inter, and
    r = inter/(area_a+area_g)  (iou = r/(1-r), monotone), then a segmented
    max and a first-index argmax via max(mask * (G-j)).
  - The per-(tile,batch) argmax is PE-transposed, broadcast to (b, j)
    partitions with selector matmuls, turned into a transposed one-hot, and
    two small accumulating matmuls against hi/mid coordinate tables gather
    the best gt box per anchor. score = rmax/(1-rmax).

The full inputs arrive here; sharding / gt-subset tables are prepared on the
host with numpy, the compiled NEFF runs SPMD on cores 0-7 via
run_bass_kernel_spmd, and outputs are un-permuted and assembled on the host.
"""

import sys
import numpy as np
from contextlib import ExitStack

for _p in ("/opt/trn_rl_repo", "/root/.axon_site/_ro/trn_rl_repo"):
    if _p not in sys.path:
        sys.path.append(_p)

from concourse import bacc, mybir, tile  # noqa: E402
from concourse.bass_utils import run_bass_kernel_spmd  # noqa: E402

F32 = mybir.dt.float32
BF16 = mybir.dt.bfloat16
AL = mybir.AluOpType
AF = mybir.ActivationFunctionType

N_CORES = 8
B = 4
NT = 104          # anchor tiles per core (A_loc = 13312 >= 12500)
NB = 13           # y-blocks per core
CH = 8            # tiles per DVE chunk
A_FULL = 100000
G_FULL = 64


def _bf16_round(x):
    """Round float32 -> bfloat16 (round-to-nearest-even), keep float32 type."""
    u = np.asarray(x, np.float32).view(np.uint32)
    rounded = (u + 0x7FFF + ((u >> 16) & 1)).astype(np.uint32) & 0xFFFF0000
    return rounded.view(np.float32)


def _to_bf16(x):
    """float32 array -> bfloat16 array (ml_dtypes if present, else uint16 bits)."""
    try:
        import ml_dtypes
        return np.asarray(x, np.float32).astype(ml_dtypes.bfloat16)
    except ImportError:
        u = np.asarray(x, np.float32).view(np.uint32)
        rounded = (u + 0x7FFF + ((u >> 16) & 1)).astype(np.uint32) & 0xFFFF0000
        return (rounded >> 16).astype(np.uint16)


def split_hml(x):
    x = np.asarray(x, np.float32)
    h = _bf16_round(x)
    m = _bf16_round(x - h)
    l = _bf16_round(x - h - m)
    return h, m, l


def build_graph(NT, B, G, CH, NB, n_cores=8):
    FB = B * G
    W = 5 * FB
    assert W <= 512
    A_loc = NT * 128
    TPB = NT // NB
    assert NT % NB == 0 and TPB % CH == 0
    NCH = NT // CH
    GP = B * G
    assert GP <= 128
    M = B * 4
    NTB = ((NT * 32 + 127) // 128) * 128 // 32
    NBLK = NTB * 32 // 128

    nc = bacc.Bacc("TRN2", target_bir_lowering=False, debug=False,
                   num_devices=n_cores)

    def din(name, shape, dt=F32):
        return nc.dram_tensor(name, list(shape), dt, kind="ExternalInput")

    lhs_d = din("lhs", [18, NT * 128], BF16)
    rhs_d = din("rhs", [18, NB * W], BF16)
    wgx_d = din("wgx", [128, NB * FB])
    wgy_d = din("wgy", [128, NB * FB])
    iot_d = din("iotarev", [128, CH * FB], BF16)
    iotp_d = din("iotarev_part", [128, 1])
    esel_d = din("esel", [128, 4 * 128], BF16)
    ident_d = din("ident", [128, 128], BF16)
    gtbl_d = din("gtbl", [128, NB * 2 * M], BF16)

    score_out = nc.dram_tensor("score", [128, NT * B], F32, kind="ExternalOutput")
    bbox_out = nc.dram_tensor("bbox", [M, A_loc], F32, kind="ExternalOutput")

    with tile.TileContext(nc) as tc, ExitStack() as ctx:
        sb = ctx.enter_context(tc.tile_pool(name="sb", bufs=1))

        rhs = sb.tile([18, NB * W], BF16)
        wgx = sb.tile([128, NB * FB], F32)
        wgy = sb.tile([128, NB * FB], F32)
        iot = sb.tile([128, CH * FB], BF16)
        iotp = sb.tile([128, 1], F32)
        esel = sb.tile([128, 4 * 128], BF16)
        ident = sb.tile([128, 128], BF16)
        gtbl = sb.tile([128, NB * 2 * M], BF16)
        for dst, src in [(rhs, rhs_d), (wgx, wgx_d), (wgy, wgy_d),
                         (iot, iot_d)]:
            nc.sync.dma_start(dst[:], src[:])

        rmax_sb = sb.tile([128, NT * B], F32)
        jrev32 = sb.tile([128, NTB * 32], BF16)
        score_sb = sb.tile([128, NT * B], F32)
        jrevT = sb.tile([128, NBLK * 128], BF16)
        oht = sb.tile([128, A_loc], BF16)
        bbs = sb.tile([M, A_loc], F32)
        nc.vector.memset(jrev32[:], 0.0)

        # ------- phase 1 + per-block transpose/one-hot/gather, fused -------
        AB = TPB * 128
        CW = 512 if AB % 512 == 0 else AB // (AB // 512 + 1)
        with tc.tile_pool(name="ps", bufs=5, space="PSUM") as ps, \
             tc.tile_pool(name="psa", bufs=1, space="PSUM") as psa, \
             tc.tile_pool(name="pst", bufs=2, space="PSUM") as pst, \
             tc.tile_pool(name="lp", bufs=3) as lp, \
             tc.tile_pool(name="wk", bufs=3) as wk:
            def emit_tail(bk):
                for tb in range(2 * bk, 2 * bk + (CH * 32) // 128):
                    tp = pst.tile([128, 128], BF16, tag="gtail")
                    nc.tensor.transpose(tp[:], jrev32[:, tb * 128:(tb + 1) * 128],
                                        ident[:])
                    nc.scalar.copy(jrevT[:, tb * 128:(tb + 1) * 128], tp[:])
                for g4 in range(bk * TPB, (bk + 1) * TPB, 4):
                    gw = min(4, NT - g4)
                    jb = pst.tile([128, 4, 128], F32, tag="gtail")
                    for k in range(gw):
                        t = g4 + k
                        tm4 = t % 4
                        nc.tensor.matmul(jb[:, k, :],
                                         esel[:, tm4 * 128:(tm4 + 1) * 128],
                                         jrevT[:, (t // 4) * 128:(t // 4 + 1) * 128],
                                         start=True, stop=True)
                    jbs = wk.tile([128, 4 * 128], BF16, tag="jbs")
                    nc.scalar.copy(jbs[0:GP, 0:gw * 128],
                                   jb[0:GP, 0:gw, :].rearrange("p a b -> p (a b)"))
                    nc.vector.tensor_scalar(
                        oht[0:GP, g4 * 128:(g4 + gw) * 128],
                        jbs[0:GP, 0:gw * 128],
                        iotp[0:GP, 0:1], None, AL.is_equal)
                for q in range(bk * AB, (bk + 1) * AB, CW):
                    bb = pst.tile([M, CW], F32, tag="gtail")
                    nc.tensor.matmul(bb[:], gtbl[0:GP, bk * 2 * M:bk * 2 * M + M],
                                     oht[0:GP, q:q + CW], start=True, stop=False)
                    nc.tensor.matmul(
                        bb[:], gtbl[0:GP, bk * 2 * M + M:(bk + 1) * 2 * M],
                        oht[0:GP, q:q + CW], start=False, stop=True)
                    nc.scalar.copy(bbs[:, q:q + CW], bb[:])
                if bk in (3, 7, 10, NB - 1):
                    prev = {3: 0, 7: 4, 10: 8, NB - 1: 11}[bk]
                    nc.sync.dma_start(
                        bbox_out[:, prev * AB:(bk + 1) * AB],
                        bbs[:, prev * AB:(bk + 1) * AB])
            for c in range(NCH):
                t0 = c * CH
                blk = t0 // TPB
                lhs = lp.tile([18, CH * 128], BF16, tag="lhs")
                nc.sync.dma_start(lhs[:], lhs_d[:, t0 * 128:(t0 + CH) * 128])
                if c == 1:
                    for dst, src in [(iotp, iotp_d), (esel, esel_d),
                                     (ident, ident_d), (gtbl, gtbl_d)]:
                        nc.sync.dma_start(dst[:], src[:])
                H = CH // 2
                uvx0 = ps.tile([128, H, 128], F32, tag="uvp")
                uvx1 = ps.tile([128, H, 128], F32, tag="uvp")
                uvy0 = ps.tile([128, H, 128], F32, tag="uvp")
                uvy1 = ps.tile([128, H, 128], F32, tag="uvp")
                uvxh = [uvx0, uvx1]
                uvyh = [uvy0, uvy1]
                area = psa.tile([128, CH, 64], F32, tag="area")
                for k in range(CH):
                    nc.tensor.matmul(uvxh[k // H][:, k % H, 0:2 * FB],
                                     lhs[:, k * 128:(k + 1) * 128],
                                     rhs[:, blk * W:blk * W + 2 * FB],
                                     start=True, stop=True)
                for k in range(CH):
                    nc.tensor.matmul(uvyh[k // H][:, k % H, 0:2 * FB],
                                     lhs[:, k * 128:(k + 1) * 128],
                                     rhs[:, blk * W + 2 * FB:blk * W + 4 * FB],
                                     start=True, stop=True)
                for k in range(CH):
                    nc.tensor.matmul(area[:, k, 0:FB],
                                     lhs[:, k * 128:(k + 1) * 128],
                                     rhs[:, blk * W + 4 * FB:blk * W + 5 * FB],
                                     start=True, stop=True)

                m0x = wk.tile([128, CH, FB], F32, tag="m0x")
                m0y = wk.tile([128, CH, FB], F32, tag="m0y")
                rx = wk.tile([128, CH, FB], F32, tag="rx")
                ry = wk.tile([128, CH, FB], F32, tag="ry")
                wgx_v = wgx[:, None, blk * FB:(blk + 1) * FB].to_broadcast(
                    (128, CH, FB))
                wgy_v = wgy[:, None, blk * FB:(blk + 1) * FB].to_broadcast(
                    (128, CH, FB))
                for hh in range(2):
                    sl = slice(hh * H, (hh + 1) * H)
                    wgx_h = wgx[:, None, blk * FB:(blk + 1) * FB].to_broadcast(
                        (128, H, FB))
                    wgy_h = wgy[:, None, blk * FB:(blk + 1) * FB].to_broadcast(
                        (128, H, FB))
                    nc.vector.tensor_tensor(m0x[:, sl, :], uvxh[hh][:, :, 0:FB],
                                            wgx_h, AL.min)
                    nc.scalar.activation(rx[:, sl, :], uvxh[hh][:, :, FB:2 * FB],
                                         AF.Relu)
                    nc.vector.tensor_tensor(m0y[:, sl, :], uvyh[hh][:, :, 0:FB],
                                            wgy_h, AL.min)
                    nc.scalar.activation(ry[:, sl, :], uvyh[hh][:, :, FB:2 * FB],
                                         AF.Relu)
                rc = wk.tile([128, CH, FB], F32, tag="rc")
                nc.vector.reciprocal_approx_fast(rc[:], area[:, :, 0:FB])

                mx = wk.tile([128, CH, FB], F32, tag="mx")
                my = wk.tile([128, CH, FB], F32, tag="my")
                nc.vector.tensor_tensor(mx[:], m0x[:], rx[:], AL.subtract)
                nc.vector.tensor_tensor(my[:], m0y[:], ry[:], AL.subtract)
                relu_my = wk.tile([128, CH, FB], F32, tag="relu_my")
                nc.scalar.activation(relu_my[:], my[:], AF.Relu)
                inter = wk.tile([128, CH, FB], F32, tag="inter")
                nc.vector.scalar_tensor_tensor(inter[:], mx[:], 0.0, relu_my[:],
                                               AL.max, AL.mult)
                r = wk.tile([128, CH, FB], F32, tag="r")
                nc.vector.tensor_tensor(r[:], inter[:], rc[:], AL.mult)

                r4 = r[:].rearrange("p c (b j) -> p c b j", j=G)
                rm = rmax_sb[:, t0 * B:(t0 + CH) * B].rearrange(
                    "p (c b) -> p c b", b=B)
                nc.vector.tensor_reduce(rm, r4, mybir.AxisListType.X, AL.max)
                mask = wk.tile([128, CH, B, G], BF16, tag="mask")
                rm_v = rm[:, :, :, None].to_broadcast((128, CH, B, G))
                nc.vector.tensor_tensor(mask[:], r4, rm_v, AL.is_ge)
                mi = wk.tile([128, CH, B, G], BF16, tag="mi")
                iot_v = iot[:].rearrange("p (c b j) -> p c b j", b=B, j=G)
                nc.vector.tensor_tensor(mi[:], mask[:], iot_v, AL.mult)
                jr = jrev32[:, t0 * 32:(t0 + CH) * 32].rearrange(
                    "p (c k) -> p c k", k=32)[:, :, 0:B]
                nc.vector.tensor_reduce(jr, mi[:], mybir.AxisListType.X, AL.max)

                # ---- per-block tail (emitted one block late: overlap) ----
                assert TPB == CH
                if blk > 1:
                    emit_tail(blk - 2)

            emit_tail(NB - 2)
            emit_tail(NB - 1)

        # ---------------- phase 2: score ----------------
        one_minus = sb.tile([128, NT * B], F32)
        nc.vector.tensor_scalar(one_minus[:], rmax_sb[:], -1.0, 1.0,
                                AL.mult, AL.add)
        recip = sb.tile([128, NT * B], F32)
        rscr = sb.tile([128, NT * B], F32)
        nc.vector.reciprocal_approx_accurate(recip[:], one_minus[:], rscr[:])
        nc.vector.tensor_tensor(score_sb[:], rmax_sb[:], recip[:], AL.mult)
        nc.sync.dma_start(score_out[:], score_sb[:])

    nc.compile()
    return nc


def host_prep(anchors_pad, gt_sub, NT, B, G, NB, CH=None):
    CH = CH or globals()['CH']
    """anchors_pad: [NT*128, 4] xywh. gt_sub: [NB, B, G, 4] xywh subset tables.
    Returns the per-core input dict (bf16 tensors as uint16 bit patterns)."""
    FB = B * G
    W = 5 * FB
    GP = B * G
    M = B * 4
    A_loc = NT * 128

    ax1 = anchors_pad[:, 0].astype(np.float32)
    ay1 = anchors_pad[:, 1].astype(np.float32)
    wa = anchors_pad[:, 2].astype(np.float32)
    ha = anchors_pad[:, 3].astype(np.float32)
    ax2 = ax1 + wa
    ay2 = ay1 + ha
    aa = wa * ha

    lhs = np.empty((18, A_loc), np.float32)
    lhs[0:3] = np.stack(split_hml(ax2))
    lhs[3:6] = np.stack(split_hml(ax1))
    lhs[6:9] = np.stack(split_hml(ay2))
    lhs[9:12] = np.stack(split_hml(ay1))
    lhs[12:15] = np.stack(split_hml(aa))
    lhs[15:18] = 1.0

    gx1 = gt_sub[..., 0].astype(np.float32)
    gy1 = gt_sub[..., 1].astype(np.float32)
    wg = gt_sub[..., 2].astype(np.float32)
    hg = gt_sub[..., 3].astype(np.float32)
    gx2 = gx1 + wg
    gy2 = gy1 + hg
    ag = wg * hg

    rhs = np.zeros((18, NB, 5, FB), np.float32)
    ones_rows = {0: slice(0, 3), 1: slice(3, 6), 2: slice(6, 9),
                 3: slice(9, 12), 4: slice(12, 15)}
    signs = {0: 1.0, 1: 1.0, 2: 1.0, 3: 1.0, 4: 1.0}
    gvals = {0: -gx1, 1: -gx1, 2: -gy1, 3: -gy1, 4: ag}
    for part in range(5):
        rhs[ones_rows[part], :, part, :] = signs[part]
        h, m, l = split_hml(gvals[part].reshape(NB, FB))
        rhs[15, :, part, :] = h
        rhs[16, :, part, :] = m
        rhs[17, :, part, :] = l
    rhs = rhs.reshape(18, NB * W)

    wgx = np.broadcast_to(wg.reshape(NB, FB)[None], (128, NB, FB)).reshape(
        128, NB * FB).copy()
    wgy = np.broadcast_to(hg.reshape(NB, FB)[None], (128, NB, FB)).reshape(
        128, NB * FB).copy()
    iotarev = np.broadcast_to((G - np.arange(G)).astype(np.float32)[None, :],
                              (B, G)).reshape(FB)
    iotarev = np.broadcast_to(np.tile(iotarev, CH), (128, CH * FB))
    iotp = np.zeros((128, 1), np.float32)
    p = np.arange(GP)
    iotp[:GP, 0] = G - (p % G)
    esel = np.zeros((128, 4 * 128), np.float32)
    for tm4 in range(4):
        esel[tm4 * 32 + p // G, tm4 * 128 + p] = 1.0
    ident = np.eye(128, dtype=np.float32)

    gxyxy = np.stack([gx1, gy1, gx2, gy2], axis=-1)  # [NB, B, G, 4]
    gh = _bf16_round(gxyxy)
    gm = _bf16_round(gxyxy - gh)
    gtbl = np.zeros((128, NB, 2, M), np.float32)
    bidx = p // G
    for b in range(B):
        sel = p[bidx == b]
        j = sel % G
        for cc in range(4):
            gtbl[sel, :, 0, b * 4 + cc] = gh[:, b, j, cc].T
            gtbl[sel, :, 1, b * 4 + cc] = gm[:, b, j, cc].T
    gtbl = gtbl.reshape(128, NB * 2 * M)

    return {"lhs": _to_bf16(lhs), "rhs": _to_bf16(rhs),
            "wgx": wgx, "wgy": wgy,
            "iotarev": _to_bf16(iotarev), "iotarev_part": iotp,
            "esel": _to_bf16(esel), "ident": _to_bf16(ident),
            "gtbl": _to_bf16(gtbl)}


def shard_inputs(anchors, gt, G):
    A = len(anchors)
    A_core = A // N_CORES
    A_loc = NT * 128
    TPB = NT // NB
    ax1 = anchors[:, 0]
    ay1 = anchors[:, 1]
    ax2 = ax1 + anchors[:, 2]
    ay2 = ay1 + anchors[:, 3]
    gx1 = gt[:, :, 0]
    gy1 = gt[:, :, 1]
    gx2 = gx1 + gt[:, :, 2]
    gy2 = gy1 + gt[:, :, 3]

    order = np.argsort(ax1, kind="stable")
    in_maps = []
    perms = []
    dummy = np.array([1e6, 1e6, 10.0, 10.0], np.float32)
    for c in range(N_CORES):
        idx = order[c * A_core:(c + 1) * A_core]
        idx = idx[np.argsort(ay1[idx], kind="stable")]
        perms.append(idx)
        pad = A_loc - len(idx)
        anchors_pad = np.concatenate(
            [anchors[idx], np.tile(dummy[None], (pad, 1))], axis=0)
        gt_sub = np.zeros((NB, B, G, 4), np.float32)
        gt_sub[..., 0] = 1e6
        gt_sub[..., 1] = 1e6
        gt_sub[..., 2] = 10.0
        gt_sub[..., 3] = 10.0
        for nb in range(NB):
            lo, hi = nb * TPB * 128, min((nb + 1) * TPB * 128, len(idx))
            if lo >= len(idx):
                continue
            bidx = idx[lo:hi]
            xlo, xhi = ax1[bidx].min(), ax2[bidx].max()
            ylo, yhi = ay1[bidx].min(), ay2[bidx].max()
            for b in range(B):
                ssel = np.flatnonzero((gx2[b] > xlo) & (gx1[b] < xhi) &
                                      (gy2[b] > ylo) & (gy1[b] < yhi))
                if len(ssel) == 0 or ssel[0] != 0:
                    ssel = np.concatenate([[0], ssel])
                assert len(ssel) <= G, f"gt subset {len(ssel)} exceeds G={G}"
                gt_sub[nb, b, :len(ssel)] = gt[b, ssel]
        in_maps.append(host_prep(anchors_pad, gt_sub, NT, B, G, NB))
    return in_maps, perms


def max_subset_size(anchors, gt):
    """Exact max gt-subset size over (core, block, batch) for this data."""
    A = len(anchors)
    A_core = A // N_CORES
    TPB = NT // NB
    ax1 = anchors[:, 0]
    ay1 = anchors[:, 1]
    ax2 = ax1 + anchors[:, 2]
    ay2 = ay1 + anchors[:, 3]
    gx1 = gt[:, :, 0]
    gy1 = gt[:, :, 1]
    gx2 = gx1 + gt[:, :, 2]
    gy2 = gy1 + gt[:, :, 3]
    order = np.argsort(ax1, kind="stable")
    mx = 1
    for c in range(N_CORES):
        idx = order[c * A_core:(c + 1) * A_core]
        idx = idx[np.argsort(ay1[idx], kind="stable")]
        for nb in range(NB):
            lo, hi = nb * TPB * 128, min((nb + 1) * TPB * 128, len(idx))
            if lo >= len(idx):
                continue
            bidx = idx[lo:hi]
            xlo, xhi = ax1[bidx].min(), ax2[bidx].max()
            ylo, yhi = ay1[bidx].min(), ay2[bidx].max()
            for b in range(B):
                ssel = (gx2[b] > xlo) & (gx1[b] < xhi) & \
                       (gy2[b] > ylo) & (gy1[b] < yhi)
                n = int(ssel.sum()) + (0 if ssel[0] else 1)
                mx = max(mx, n)
    return mx


_GRAPH_CACHE = {}


def _get_graph(G):
    if G not in _GRAPH_CACHE:
        _GRAPH_CACHE[G] = build_graph(NT, B, G, CH, NB, n_cores=N_CORES)
    return _GRAPH_CACHE[G]


def kernel(fs_proposal=None, ss_proposal=None, anchors=None, ground_truth=None,
           **_unused):
    anchors = np.ascontiguousarray(np.asarray(anchors, np.float32))
    gt = np.ascontiguousarray(np.asarray(ground_truth, np.float32))
    assert anchors.shape == (A_FULL, 4) and gt.shape == (B, G_FULL, 4)

    G = max(8, min(25, max_subset_size(anchors, gt)))
    nc = _get_graph(G)
    in_maps, perms = shard_inputs(anchors, gt, G)
    res = run_bass_kernel_spmd(nc, in_maps, core_ids=list(range(N_CORES)))

    A_loc = NT * 128
    score = np.empty((B, A_FULL), np.float32)
    bbox = np.empty((B, A_FULL, 4), np.float32)
    for c in range(N_CORES):
        idx = perms[c]
        n = len(idx)
        sc = res.results[c]["score"].reshape(128, NT, B).transpose(2, 1, 0)
        score[:, idx] = sc.reshape(B, A_loc)[:, :n]
        bb = res.results[c]["bbox"].reshape(B, 4, A_loc)
        bbox[:, idx, :] = bb[:, :, :n].transpose(0, 2, 1)
    return score, bbox


# revision 22
# speedup vs baseline: 1.5029x; 1.0026x over previous
"""Trainium2 Bass kernel for nn_AInnoFaceLoss (IoU -> argmax -> gather).

reference semantics:
    a = xyxy(anchors); g = xyxy(ground_truth)
    iou[b, i, j] over (A=100000 anchors) x (G=64 gt) per batch (B=4)
    target_score = iou.max(-1); target_bbox = g[b, argmax(-1)]
(fs_proposal / ss_proposal are unused by the reference.)

Strategy (8 NeuronCores, data-parallel over anchors):
  - Host sorts anchors by x into 8 strips (one per core); within a strip,
    anchors are sorted by y into NB blocks. Each (core, block, batch) only
    ever overlaps a small subset of the 64 gt boxes (computed exactly on the
    host; global gt 0 is always included so all-zero-IoU rows reproduce the
    reference argmax tie-break). Subsets are padded to G_PAD with far-away
    dummy boxes.
  - Per anchor-tile (128 anchors) one bf16 matmul with hi/mid/lo split
    operands (exact to fp32) emits [Ux|Vx|Uy|Vy|AREA] where
    U = ax2-gx1, V = gx2-ax1 (per axis) and AREA = area_a + area_g.
  - DVE computes inter_w = relu(min(U, wa, V, wg)) per axis, inter, and
    r = inter/(area_a+area_g)  (iou = r/(1-r), monotone), then a segmented
    max and a first-index argmax via max(mask * (G-j)).
  - The per-(tile,batch) argmax is PE-transposed, broadcast to (b, j)
    partitions with selector matmuls, turned into a transposed one-hot, and
    two small accumulating matmuls against hi/mid coordinate tables gather
    the best gt box per anchor. score = rmax/(1-rmax).

The full inputs arrive here; sharding / gt-subset tables are prepared on the
host with numpy, the compiled NEFF runs SPMD on cores 0-7 via
run_bass_kernel_spmd, and outputs are un-permuted and assembled on the host.
"""

import sys
import numpy as np
from contextlib import ExitStack

for _p in ("/opt/trn_rl_repo", "/root/.axon_site/_ro/trn_rl_repo"):
    if _p not in sys.path:
        sys.path.append(_p)

from concourse import bacc, mybir, tile  # noqa: E402
from concourse.bass_utils import run_bass_kernel_spmd  # noqa: E402

F32 = mybir.dt.float32
BF16 = mybir.dt.bfloat16
AL = mybir.AluOpType
AF = mybir.ActivationFunctionType

N_CORES = 8
B = 4
NT = 104          # anchor tiles per core (A_loc = 13312 >= 12500)
NB = 13           # y-blocks per core
CH = 8            # tiles per DVE chunk
A_FULL = 100000
G_FULL = 64


def _bf16_round(x):
    """Round float32 -> bfloat16 (round-to-nearest-even), keep float32 type."""
    u = np.asarray(x, np.float32).view(np.uint32)
    rounded = (u + 0x7FFF + ((u >> 16) & 1)).astype(np.uint32) & 0xFFFF0000
    return rounded.view(np.float32)


def _to_bf16(x):
    """float32 array -> bfloat16 array (ml_dtypes if present, else uint16 bits)."""
    try:
        import ml_dtypes
        return np.asarray(x, np.float32).astype(ml_dtypes.bfloat16)
    except ImportError:
        u = np.asarray(x, np.float32).view(np.uint32)
        rounded = (u + 0x7FFF + ((u >> 16) & 1)).astype(np.uint32) & 0xFFFF0000
        return (rounded >> 16).astype(np.uint16)


def split_hml(x):
    x = np.asarray(x, np.float32)
    h = _bf16_round(x)
    m = _bf16_round(x - h)
    l = _bf16_round(x - h - m)
    return h, m, l


def build_graph(NT, B, G, CH, NB, n_cores=8):
    FB = B * G
    W = 5 * FB
    assert W <= 512
    A_loc = NT * 128
    TPB = NT // NB
    assert NT % NB == 0 and TPB % CH == 0
    NCH = NT // CH
    GP = B * G
    assert GP <= 128
    M = B * 4
    NTB = ((NT * 32 + 127) // 128) * 128 // 32
    NBLK = NTB * 32 // 128

    nc = bacc.Bacc("TRN2", target_bir_lowering=False, debug=False,
                   num_devices=n_cores)

    def din(name, shape, dt=F32):
        return nc.dram_tensor(name, list(shape), dt, kind="ExternalInput")

    lhs_d = din("lhs", [18, NT * 128], BF16)
    rhs_d = din("rhs", [18, NB * W], BF16)
    wgx_d = din("wgx", [128, NB * FB])
    wgy_d = din("wgy", [128, NB * FB])
    iot_d = din("iotarev", [128, CH * FB], BF16)
    iotp_d = din("iotarev_part", [128, 1])
    esel_d = din("esel", [128, 4 * 128], BF16)
    ident_d = din("ident", [128, 128], BF16)
    gtbl_d = din("gtbl", [128, NB * 2 * M], BF16)

    score_out = nc.dram_tensor("score", [128, NT * B], F32, kind="ExternalOutput")
    bbox_out = nc.dram_tensor("bbox", [M, A_loc], F32, kind="ExternalOutput")

    with tile.TileContext(nc) as tc, ExitStack() as ctx:
        sb = ctx.enter_context(tc.tile_pool(name="sb", bufs=1))

        rhs = sb.tile([18, NB * W], BF16)
        wgx = sb.tile([128, NB * FB], F32)
        wgy = sb.tile([128, NB * FB], F32)
        iot = sb.tile([128, CH * FB], BF16)
        iotp = sb.tile([128, 1], F32)
        esel = sb.tile([128, 4 * 128], BF16)
        ident = sb.tile([128, 128], BF16)
        gtbl = sb.tile([128, NB * 2 * M], BF16)
        for dst, src in [(rhs, rhs_d), (wgx, wgx_d), (wgy, wgy_d),
                         (iot, iot_d)]:
            nc.sync.dma_start(dst[:], src[:])

        rmax_sb = sb.tile([128, NT * B], F32)
        jrev32 = sb.tile([128, NTB * 32], BF16)
        score_sb = sb.tile([128, NT * B], F32)
        jrevT = sb.tile([128, NBLK * 128], BF16)
        oht = sb.tile([128, A_loc], BF16)
        bbs = sb.tile([M, A_loc], F32)
        nc.vector.memset(jrev32[:], 0.0)

        # ------- phase 1 + per-block transpose/one-hot/gather, fused -------
        AB = TPB * 128
        CW = 512 if AB % 512 == 0 else AB // (AB // 512 + 1)
        with tc.tile_pool(name="ps", bufs=4, space="PSUM") as ps, \
             tc.tile_pool(name="psa", bufs=2, space="PSUM") as psa, \
             tc.tile_pool(name="pst", bufs=2, space="PSUM") as pst, \
             tc.tile_pool(name="lp", bufs=3) as lp, \
             tc.tile_pool(name="wk", bufs=3) as wk:
            def emit_tail(bk):
                for tb in range(2 * bk, 2 * bk + (CH * 32) // 128):
                    tp = pst.tile([128, 128], BF16, tag="gtail")
                    nc.tensor.transpose(tp[:], jrev32[:, tb * 128:(tb + 1) * 128],
                                        ident[:])
                    nc.scalar.copy(jrevT[:, tb * 128:(tb + 1) * 128], tp[:])
                for g4 in range(bk * TPB, (bk + 1) * TPB, 4):
                    gw = min(4, NT - g4)
                    jb = pst.tile([128, 4, 128], F32, tag="gtail")
                    for k in range(gw):
                        t = g4 + k
                        tm4 = t % 4
                        nc.tensor.matmul(jb[:, k, :],
                                         esel[:, tm4 * 128:(tm4 + 1) * 128],
                                         jrevT[:, (t // 4) * 128:(t // 4 + 1) * 128],
                                         start=True, stop=True)
                    jbs = wk.tile([128, 4 * 128], BF16, tag="jbs")
                    nc.scalar.copy(jbs[0:GP, 0:gw * 128],
                                   jb[0:GP, 0:gw, :].rearrange("p a b -> p (a b)"))
                    nc.vector.tensor_scalar(
                        oht[0:GP, g4 * 128:(g4 + gw) * 128],
                        jbs[0:GP, 0:gw * 128],
                        iotp[0:GP, 0:1], None, AL.is_equal)
                for q in range(bk * AB, (bk + 1) * AB, CW):
                    bb = pst.tile([M, CW], F32, tag="gtail")
                    nc.tensor.matmul(bb[:], gtbl[0:GP, bk * 2 * M:bk * 2 * M + M],
                                     oht[0:GP, q:q + CW], start=True, stop=False)
                    nc.tensor.matmul(
                        bb[:], gtbl[0:GP, bk * 2 * M + M:(bk + 1) * 2 * M],
                        oht[0:GP, q:q + CW], start=False, stop=True)
                    nc.scalar.copy(bbs[:, q:q + CW], bb[:])
                if bk in (3, 7, 10, NB - 1):
                    prev = {3: 0, 7: 4, 10: 8, NB - 1: 11}[bk]
                    nc.sync.dma_start(
                        bbox_out[:, prev * AB:(bk + 1) * AB],
                        bbs[:, prev * AB:(bk + 1) * AB])
            for c in range(NCH):
                t0 = c * CH
                blk = t0 // TPB
                lhs = lp.tile([18, CH * 128], BF16, tag="lhs")
                nc.sync.dma_start(lhs[:], lhs_d[:, t0 * 128:(t0 + CH) * 128])
                if c == 1:
                    for dst, src in [(iotp, iotp_d), (esel, esel_d),
                                     (ident, ident_d), (gtbl, gtbl_d)]:
                        nc.sync.dma_start(dst[:], src[:])
                H = CH // 2
                uvx0 = ps.tile([128, H, 128], F32, tag="uvp")
                uvx1 = ps.tile([128, H, 128], F32, tag="uvp")
                uvy0 = ps.tile([128, H, 128], F32, tag="uvp")
                uvy1 = ps.tile([128, H, 128], F32, tag="uvp")
                uvxh = [uvx0, uvx1]
                uvyh = [uvy0, uvy1]
                area = psa.tile([128, CH, 64], F32, tag="area")
                for k in range(CH):
                    nc.tensor.matmul(uvxh[k // H][:, k % H, 0:2 * FB],
                                     lhs[:, k * 128:(k + 1) * 128],
                                     rhs[:, blk * W:blk * W + 2 * FB],
                                     start=True, stop=True)
                for k in range(CH):
                    nc.tensor.matmul(uvyh[k // H][:, k % H, 0:2 * FB],
                                     lhs[:, k * 128:(k + 1) * 128],
                                     rhs[:, blk * W + 2 * FB:blk * W + 4 * FB],
                                     start=True, stop=True)
                for k in range(CH):
                    nc.tensor.matmul(area[:, k, 0:FB],
                                     lhs[:, k * 128:(k + 1) * 128],
                                     rhs[:, blk * W + 4 * FB:blk * W + 5 * FB],
                                     start=True, stop=True)

                m0x = wk.tile([128, CH, FB], F32, tag="m0x")
                m0y = wk.tile([128, CH, FB], F32, tag="m0y")
                rx = wk.tile([128, CH, FB], F32, tag="rx")
                ry = wk.tile([128, CH, FB], F32, tag="ry")
                wgx_v = wgx[:, None, blk * FB:(blk + 1) * FB].to_broadcast(
                    (128, CH, FB))
                wgy_v = wgy[:, None, blk * FB:(blk + 1) * FB].to_broadcast(
                    (128, CH, FB))
                for hh in range(2):
                    sl = slice(hh * H, (hh + 1) * H)
                    wgx_h = wgx[:, None, blk * FB:(blk + 1) * FB].to_broadcast(
                        (128, H, FB))
                    wgy_h = wgy[:, None, blk * FB:(blk + 1) * FB].to_broadcast(
                        (128, H, FB))
                    nc.vector.tensor_tensor(m0x[:, sl, :], uvxh[hh][:, :, 0:FB],
                                            wgx_h, AL.min)
                    nc.scalar.activation(rx[:, sl, :], uvxh[hh][:, :, FB:2 * FB],
                                         AF.Relu)
                    nc.vector.tensor_tensor(m0y[:, sl, :], uvyh[hh][:, :, 0:FB],
                                            wgy_h, AL.min)
                    nc.scalar.activation(ry[:, sl, :], uvyh[hh][:, :, FB:2 * FB],
                                         AF.Relu)
                rc = wk.tile([128, CH, FB], F32, tag="rc")
                nc.vector.reciprocal_approx_fast(rc[:], area[:, :, 0:FB])

                mx = wk.tile([128, CH, FB], F32, tag="mx")
                my = wk.tile([128, CH, FB], F32, tag="my")
                nc.vector.tensor_tensor(mx[:], m0x[:], rx[:], AL.subtract)
                nc.vector.tensor_tensor(my[:], m0y[:], ry[:], AL.subtract)
                relu_my = wk.tile([128, CH, FB], F32, tag="relu_my")
                nc.scalar.activation(relu_my[:], my[:], AF.Relu)
                inter = wk.tile([128, CH, FB], F32, tag="inter")
                nc.vector.scalar_tensor_tensor(inter[:], mx[:], 0.0, relu_my[:],
                                               AL.max, AL.mult)
                r = wk.tile([128, CH, FB], F32, tag="r")
                nc.vector.tensor_tensor(r[:], inter[:], rc[:], AL.mult)

                r4 = r[:].rearrange("p c (b j) -> p c b j", j=G)
                rm = rmax_sb[:, t0 * B:(t0 + CH) * B].rearrange(
                    "p (c b) -> p c b", b=B)
                nc.vector.tensor_reduce(rm, r4, mybir.AxisListType.X, AL.max)
                mask = wk.tile([128, CH, B, G], BF16, tag="mask")
                rm_v = rm[:, :, :, None].to_broadcast((128, CH, B, G))
                nc.vector.tensor_tensor(mask[:], r4, rm_v, AL.is_ge)
                mi = wk.tile([128, CH, B, G], BF16, tag="mi")
                iot_v = iot[:].rearrange("p (c b j) -> p c b j", b=B, j=G)
                nc.vector.tensor_tensor(mi[:], mask[:], iot_v, AL.mult)
                jr = jrev32[:, t0 * 32:(t0 + CH) * 32].rearrange(
                    "p (c k) -> p c k", k=32)[:, :, 0:B]
                nc.vector.tensor_reduce(jr, mi[:], mybir.AxisListType.X, AL.max)

                # ---- per-block tail (emitted one block late: overlap) ----
                assert TPB == CH
                if blk > 1:
                    emit_tail(blk - 2)

            emit_tail(NB - 2)
            emit_tail(NB - 1)

        # ---------------- phase 2: score ----------------
        one_minus = sb.tile([128, NT * B], F32)
        nc.vector.tensor_scalar(one_minus[:], rmax_sb[:], -1.0, 1.0,
                                AL.mult, AL.add)
        recip = sb.tile([128, NT * B], F32)
        rscr = sb.tile([128, NT * B], F32)
        nc.vector.reciprocal_approx_accurate(recip[:], one_minus[:], rscr[:])
        nc.vector.tensor_tensor(score_sb[:], rmax_sb[:], recip[:], AL.mult)
        nc.sync.dma_start(score_out[:], score_sb[:])

    nc.compile()
    return nc


def host_prep(anchors_pad, gt_sub, NT, B, G, NB, CH=None):
    CH = CH or globals()['CH']
    """anchors_pad: [NT*128, 4] xywh. gt_sub: [NB, B, G, 4] xywh subset tables.
    Returns the per-core input dict (bf16 tensors as uint16 bit patterns)."""
    FB = B * G
    W = 5 * FB
    GP = B * G
    M = B * 4
    A_loc = NT * 128

    ax1 = anchors_pad[:, 0].astype(np.float32)
    ay1 = anchors_pad[:, 1].astype(np.float32)
    wa = anchors_pad[:, 2].astype(np.float32)
    ha = anchors_pad[:, 3].astype(np.float32)
    ax2 = ax1 + wa
    ay2 = ay1 + ha
    aa = wa * ha

    lhs = np.empty((18, A_loc), np.float32)
    lhs[0:3] = np.stack(split_hml(ax2))
    lhs[3:6] = np.stack(split_hml(ax1))
    lhs[6:9] = np.stack(split_hml(ay2))
    lhs[9:12] = np.stack(split_hml(ay1))
    lhs[12:15] = np.stack(split_hml(aa))
    lhs[15:18] = 1.0

    gx1 = gt_sub[..., 0].astype(np.float32)
    gy1 = gt_sub[..., 1].astype(np.float32)
    wg = gt_sub[..., 2].astype(np.float32)
    hg = gt_sub[..., 3].astype(np.float32)
    gx2 = gx1 + wg
    gy2 = gy1 + hg
    ag = wg * hg

    rhs = np.zeros((18, NB, 5, FB), np.float32)
    ones_rows = {0: slice(0, 3), 1: slice(3, 6), 2: slice(6, 9),
                 3: slice(9, 12), 4: slice(12, 15)}
    signs = {0: 1.0, 1: 1.0, 2: 1.0, 3: 1.0, 4: 1.0}
    gvals = {0: -gx1, 1: -gx1, 2: -gy1, 3: -gy1, 4: ag}
    for part in range(5):
        rhs[ones_rows[part], :, part, :] = signs[part]
        h, m, l = split_hml(gvals[part].reshape(NB, FB))
        rhs[15, :, part, :] = h
        rhs[16, :, part, :] = m
        rhs[17, :, part, :] = l
    rhs = rhs.reshape(18, NB * W)

    wgx = np.broadcast_to(wg.reshape(NB, FB)[None], (128, NB, FB)).reshape(
        128, NB * FB).copy()
    wgy = np.broadcast_to(hg.reshape(NB, FB)[None], (128, NB, FB)).reshape(
        128, NB * FB).copy()
    iotarev = np.broadcast_to((G - np.arange(G)).astype(np.float32)[None, :],
                              (B, G)).reshape(FB)
    iotarev = np.broadcast_to(np.tile(iotarev, CH), (128, CH * FB))
    iotp = np.zeros((128, 1), np.float32)
    p = np.arange(GP)
    iotp[:GP, 0] = G - (p % G)
    esel = np.zeros((128, 4 * 128), np.float32)
    for tm4 in range(4):
        esel[tm4 * 32 + p // G, tm4 * 128 + p] = 1.0
    ident = np.eye(128, dtype=np.float32)

    gxyxy = np.stack([gx1, gy1, gx2, gy2], axis=-1)  # [NB, B, G, 4]
    gh = _bf16_round(gxyxy)
    gm = _bf16_round(gxyxy - gh)
    gtbl = np.zeros((128, NB, 2, M), np.float32)
    bidx = p // G
    for b in range(B):
        sel = p[bidx == b]
        j = sel % G
        for cc in range(4):
            gtbl[sel, :, 0, b * 4 + cc] = gh[:, b, j, cc].T
            gtbl[sel, :, 1, b * 4 + cc] = gm[:, b, j, cc].T
    gtbl = gtbl.reshape(128, NB * 2 * M)

    return {"lhs": _to_bf16(lhs), "rhs": _to_bf16(rhs),
            "wgx": wgx, "wgy": wgy,
            "iotarev": _to_bf16(iotarev), "iotarev_part": iotp,
            "esel": _to_bf16(esel), "ident": _to_bf16(ident),
            "gtbl": _to_bf16(gtbl)}


def shard_inputs(anchors, gt, G):
    A = len(anchors)
    A_core = A // N_CORES
    A_loc = NT * 128
    TPB = NT // NB
    ax1 = anchors[:, 0]
    ay1 = anchors[:, 1]
    ax2 = ax1 + anchors[:, 2]
    ay2 = ay1 + anchors[:, 3]
    gx1 = gt[:, :, 0]
    gy1 = gt[:, :, 1]
    gx2 = gx1 + gt[:, :, 2]
    gy2 = gy1 + gt[:, :, 3]

    order = np.argsort(ax1, kind="stable")
    in_maps = []
    perms = []
    dummy = np.array([1e6, 1e6, 10.0, 10.0], np.float32)
    for c in range(N_CORES):
        idx = order[c * A_core:(c + 1) * A_core]
        idx = idx[np.argsort(ay1[idx], kind="stable")]
        perms.append(idx)
        pad = A_loc - len(idx)
        anchors_pad = np.concatenate(
            [anchors[idx], np.tile(dummy[None], (pad, 1))], axis=0)
        gt_sub = np.zeros((NB, B, G, 4), np.float32)
        gt_sub[..., 0] = 1e6
        gt_sub[..., 1] = 1e6
        gt_sub[..., 2] = 10.0
        gt_sub[..., 3] = 10.0
        for nb in range(NB):
            lo, hi = nb * TPB * 128, min((nb + 1) * TPB * 128, len(idx))
            if lo >= len(idx):
                continue
            bidx = idx[lo:hi]
            xlo, xhi = ax1[bidx].min(), ax2[bidx].max()
            ylo, yhi = ay1[bidx].min(), ay2[bidx].max()
            for b in range(B):
                ssel = np.flatnonzero((gx2[b] > xlo) & (gx1[b] < xhi) &
                                      (gy2[b] > ylo) & (gy1[b] < yhi))
                if len(ssel) == 0 or ssel[0] != 0:
                    ssel = np.concatenate([[0], ssel])
                assert len(ssel) <= G, f"gt subset {len(ssel)} exceeds G={G}"
                gt_sub[nb, b, :len(ssel)] = gt[b, ssel]
        in_maps.append(host_prep(anchors_pad, gt_sub, NT, B, G, NB))
    return in_maps, perms


def max_subset_size(anchors, gt):
    """Exact max gt-subset size over (core, block, batch) for this data."""
    A = len(anchors)
    A_core = A // N_CORES
    TPB = NT // NB
    ax1 = anchors[:, 0]
    ay1 = anchors[:, 1]
    ax2 = ax1 + anchors[:, 2]
    ay2 = ay1 + anchors[:, 3]
    gx1 = gt[:, :, 0]
    gy1 = gt[:, :, 1]
    gx2 = gx1 + gt[:, :, 2]
    gy2 = gy1 + gt[:, :, 3]
    order = np.argsort(ax1, kind="stable")
    mx = 1
    for c in range(N_CORES):
        idx = order[c * A_core:(c + 1) * A_core]
        idx = idx[np.argsort(ay1[idx], kind="stable")]
        for nb in range(NB):
            lo, hi = nb * TPB * 128, min((nb + 1) * TPB * 128, len(idx))
            if lo >= len(idx):
                continue
            bidx = idx[lo:hi]
            xlo, xhi = ax1[bidx].min(), ax2[bidx].max()
            ylo, yhi = ay1[bidx].min(), ay2[bidx].max()
            for b in range(B):
                ssel = (gx2[b] > xlo) & (gx1[b] < xhi) & \
                       (gy2[b] > ylo) & (gy1[b] < yhi)
                n = int(ssel.sum()) + (0 if ssel[0] else 1)
                mx = max(mx, n)
    return mx


_GRAPH_CACHE = {}


def _get_graph(G):
    if G not in _GRAPH_CACHE:
        _GRAPH_CACHE[G] = build_graph(NT, B, G, CH, NB, n_cores=N_CORES)
    return _GRAPH_CACHE[G]


def kernel(fs_proposal=None, ss_proposal=None, anchors=None, ground_truth=None,
           **_unused):
    anchors = np.ascontiguousarray(np.asarray(anchors, np.float32))
    gt = np.ascontiguousarray(np.asarray(ground_truth, np.float32))
    assert anchors.shape == (A_FULL, 4) and gt.shape == (B, G_FULL, 4)

    G = max(8, min(25, max_subset_size(anchors, gt)))
    nc = _get_graph(G)
    in_maps, perms = shard_inputs(anchors, gt, G)
    res = run_bass_kernel_spmd(nc, in_maps, core_ids=list(range(N_CORES)))

    A_loc = NT * 128
    score = np.empty((B, A_FULL), np.float32)
    bbox = np.empty((B, A_FULL, 4), np.float32)
    for c in range(N_CORES):
        idx = perms[c]
        n = len(idx)
        sc = res.results[c]["score"].reshape(128, NT, B).transpose(2, 1, 0)
        score[:, idx] = sc.reshape(B, A_loc)[:, :n]
        bb = res.results[c]["bbox"].reshape(B, 4, A_loc)
        bbox[:, idx, :] = bb[:, :, :n].transpose(0, 2, 1)
    return score, bbox
